# revision 7
# baseline (speedup 1.0000x reference)
"""Trainium2 Bass kernel for the instant-NGP Density network (hash-grid encode + MLP).

Strategy: data-parallel over points (8 NeuronCores, 131072 points each).
Per core, a For_i hardware loop encodes chunks of points:
  - DVE computes the 24-level spatial-hash indices with exact integer math
    (fp32-exact split multiplies + int32 xor/and/shift ops),
  - per-corner features are gathered from the HBM-resident hash table via
    indirect DMAs (128 offsets per instruction, 8 bytes per fetch),
  - trilinear interpolation accumulates the 48-dim encoding, staged to DRAM.
A second loop runs the 48->64->64->33 MLP on the tensor engine (PE transposes
to feature-major, fp32 matmuls, ReLU+bias on ACT, softplus on the density
column) and writes the [N, 33] result.
"""
import sys

sys.path.insert(0, "/opt/trn_rl_repo")

import numpy as np

import concourse.bass_utils as _bu

# walrus needs these DGE levels for vector-indirect (gather) DMAs; without
# them the indirect descriptors are compiled wrong.
_orig_get_walrus_args = _bu.get_walrus_args
def _patched_get_walrus_args(arch, tmpdir, *, dve_root=None):
    return [
        "--dge-levels=io,scalar_dynamic_offset,vector_dynamic_offsets,dynamic_size,dst_reduce"
    ] + _orig_get_walrus_args(arch, tmpdir, dve_root=dve_root)
_bu.get_walrus_args = _patched_get_walrus_args

import concourse.bass as bass
import concourse.bacc as bacc
import concourse.mybir as mybir
import concourse.tile as tile
from concourse.bass import ts
from concourse.bass_utils import run_bass_kernel_spmd

# ---- problem constants ----
L = 24
F = 2
T = 2 ** 19
M = T - 1
BASE_RES = 16
FINEST_RES = 2048
_b = np.exp(np.log(FINEST_RES / BASE_RES) / (L - 1))
RES = np.floor(BASE_RES * _b ** np.arange(L)).astype(np.float32)
P2M = 2654435761 % T
P3M = 805459861 % T
D1_P2, D0_P2 = P2M >> 7, P2M & 127
D1_P3, D0_P3 = P3M >> 7, P3M & 127

P = 128
CH = 8
N_CORES = 8
FDT = mybir.dt.float32
IDT = mybir.dt.int32
AO = mybir.AluOpType

_CACHE = {}


def _encode_phase(nc, tc, cols, pts_d, tbl_d, res_d, lvb_d, enc_d):
    n_iter = cols // CH
    with (
        tc.tile_pool(name="cst", bufs=1) as cst,
        tc.tile_pool(name="hsh", bufs=2) as hsh,
        tc.tile_pool(name="gth", bufs=2) as gth,
    ):
        res_t = cst.tile([P, L], FDT, tag="resc")
        nc.sync.dma_start(out=res_t[:], in_=res_d[:])
        lvb_t = cst.tile([P, L], IDT, tag="lvbc")
        nc.sync.dma_start(out=lvb_t[:], in_=lvb_d[:])

        def body(it):
            p_t = hsh.tile([P, CH, 3], FDT, tag="p_t")
            nc.sync.dma_start(out=p_t[:], in_=pts_d[:, ts(it, CH), :])
            xn = hsh.tile([P, CH, 3], FDT, tag="xn")
            nc.vector.tensor_scalar(out=xn[:], in0=p_t[:], scalar1=0.5,
                                    scalar2=0.5, op0=AO.mult, op1=AO.add)
            nc.vector.tensor_scalar(out=xn[:], in0=xn[:], scalar1=0.0,
                                    scalar2=1.0, op0=AO.max, op1=AO.min)

            pos, i0f, i0i, wgt = [], [], [], []
            for a in range(3):
                pa = hsh.tile([P, CH, L], FDT, tag=f"pos{a}")
                nc.vector.tensor_tensor(
                    out=pa[:], in0=xn[:, :, a].to_broadcast([P, CH, L]),
                    in1=res_t[:].unsqueeze(1).to_broadcast([P, CH, L]), op=AO.mult)
                ii = hsh.tile([P, CH, L], IDT, tag=f"i0i{a}")
                nc.vector.tensor_copy(out=ii[:], in_=pa[:])   # trunc toward 0
                ff = hsh.tile([P, CH, L], FDT, tag=f"i0f{a}")
                nc.vector.tensor_copy(out=ff[:], in_=ii[:])
                ww = hsh.tile([P, CH, L], FDT, tag=f"wgt{a}")
                nc.vector.tensor_tensor(out=ww[:], in0=pa[:], in1=ff[:],
                                        op=AO.subtract)
                pos.append(pa); i0i.append(ii); i0f.append(ff); wgt.append(ww)

            def hash_axis(a, d1, d0, pm):
                m1 = hsh.tile([P, CH, L], FDT, tag=f"m1{a}")
                nc.vector.tensor_scalar_mul(out=m1[:], in0=i0f[a][:], scalar1=float(d1))
                m1i = hsh.tile([P, CH, L], IDT, tag=f"m1i{a}")
                nc.vector.tensor_copy(out=m1i[:], in_=m1[:])
                tt = hsh.tile([P, CH, L], IDT, tag=f"tt{a}")
                nc.vector.tensor_scalar(out=tt[:], in0=m1i[:], scalar1=0xFFF,
                                        scalar2=7, op0=AO.bitwise_and,
                                        op1=AO.logical_shift_left)
                m0 = hsh.tile([P, CH, L], FDT, tag=f"m0{a}")
                nc.vector.tensor_scalar_mul(out=m0[:], in0=i0f[a][:], scalar1=float(d0))
                m0i = hsh.tile([P, CH, L], IDT, tag=f"m0i{a}")
                nc.vector.tensor_copy(out=m0i[:], in_=m0[:])
                s = hsh.tile([P, CH, L], IDT, tag=f"s{a}")
                nc.vector.tensor_tensor(out=s[:], in0=tt[:], in1=m0i[:], op=AO.add)
                h0 = hsh.tile([P, CH, L], IDT, tag=f"h0{a}")
                nc.vector.tensor_scalar(out=h0[:], in0=s[:], scalar1=M,
                                        scalar2=None, op0=AO.bitwise_and)
                h1 = hsh.tile([P, CH, L], IDT, tag=f"h1{a}")
                nc.vector.tensor_scalar_add(out=h1[:], in0=s[:], scalar1=int(pm))
                nc.vector.tensor_scalar(out=h1[:], in0=h1[:], scalar1=M,
                                        scalar2=None, op0=AO.bitwise_and)
                return h0, h1

            hy0, hy1 = hash_axis(1, D1_P2, D0_P2, P2M)
            hz0, hz1 = hash_axis(2, D1_P3, D0_P3, P3M)
            for hz in (hz0, hz1):
                nc.vector.tensor_tensor(out=hz[:], in0=hz[:],
                                        in1=lvb_t[:].unsqueeze(1).to_broadcast([P, CH, L]),
                                        op=AO.add)
            x1i = hsh.tile([P, CH, L], IDT, tag="x1i")
            nc.vector.tensor_scalar_add(out=x1i[:], in0=i0i[0][:], scalar1=1)

            syz = []
            for bz, hz in enumerate((hz0, hz1)):
                for by, hy in enumerate((hy0, hy1)):
                    s = hsh.tile([P, CH, L], IDT, tag=f"syz{bz}{by}")
                    nc.vector.tensor_tensor(out=s[:], in0=hy[:], in1=hz[:],
                                            op=AO.bitwise_xor)
                    syz.append(s)   # index bz*2+by

            idx_t = gth.tile([P, CH, L, 2, 2, 2], IDT, tag="idx")
            for c in range(8):
                bx, by, bz = c & 1, (c >> 1) & 1, (c >> 2) & 1
                cx = x1i if bx else i0i[0]
                nc.vector.tensor_tensor(out=idx_t[:, :, :, bz, by, bx],
                                        in0=syz[bz * 2 + by][:], in1=cx[:],
                                        op=AO.bitwise_xor)

            feat = gth.tile([P, CH, L, 2, 2, 2, F], FDT, tag="feat")
            for i in range(CH):
                for lv in range(L):
                    for c in range(8):
                        bx, by, bz = c & 1, (c >> 1) & 1, (c >> 2) & 1
                        nc.gpsimd.indirect_dma_start(
                            out=feat[:, i, lv, bz, by, bx, :], out_offset=None,
                            in_=tbl_d[:],
                            in_offset=bass.IndirectOffsetOnAxis(
                                ap=idx_t[:, i, lv, bz, by, bx:bx + 1], axis=0))

            # expand lerp weights densely (zero-step dims don't merge in APs)
            wxe = hsh.tile([P, CH, L, 4], FDT, tag="wxe")
            nc.vector.tensor_copy(out=wxe[:], in_=wgt[0][:].to_broadcast([P, CH, L, 4]))
            wye = hsh.tile([P, CH, L, 2], FDT, tag="wye")
            nc.vector.tensor_copy(out=wye[:], in_=wgt[1][:].to_broadcast([P, CH, L, 2]))

            def lerp(even_ap, odd_ap, out_ap, w_ap, shape, tag):
                d = hsh.tile(shape, FDT, tag=tag)
                nc.vector.tensor_tensor(out=d[:], in0=odd_ap, in1=even_ap,
                                        op=AO.subtract)
                nc.vector.tensor_tensor(out=d[:], in0=d[:], in1=w_ap, op=AO.mult)
                nc.vector.tensor_tensor(out=out_ap, in0=d[:], in1=even_ap,
                                        op=AO.add)

            fx = hsh.tile([P, CH, L, 2, 2, F], FDT, tag="fx")
            lerp(feat[:, :, :, :, :, 0, :], feat[:, :, :, :, :, 1, :], fx[:],
                 wxe[:].rearrange("p i l (a b) -> p i l a b", a=2, b=2)
                       .to_broadcast([P, CH, L, 2, 2, F]),
                 [P, CH, L, 2, 2, F], "dx")
            fy = hsh.tile([P, CH, L, 2, F], FDT, tag="fy")
            lerp(fx[:, :, :, :, 0, :], fx[:, :, :, :, 1, :], fy[:],
                 wye[:].to_broadcast([P, CH, L, 2, F]),
                 [P, CH, L, 2, F], "dy")
            enc_t = hsh.tile([P, CH, L, F], FDT, tag="enc")
            lerp(fy[:, :, :, 0, :], fy[:, :, :, 1, :], enc_t[:],
                 wgt[2][:].to_broadcast([P, CH, L, F]),
                 [P, CH, L, F], "dz")
            nc.sync.dma_start(out=enc_d[:, ts(it, CH), :],
                              in_=enc_t[:].rearrange("p i l f -> p i (l f)"))

        with tc.For_i(0, n_iter, 1) as it:
            body(it)


def _mlp_phase(nc, tc, cols, enc_d, w_d, b_d, ident_d, out_d):
    ch2 = min(128, cols)
    n2 = cols // ch2
    NT = 4  # 4 x 128 = 512-wide matmul tiles
    with (
        tc.tile_pool(name="wts", bufs=1) as wts,
        tc.tile_pool(name="mlp", bufs=2) as mlp,
        tc.tile_pool(name="psA", bufs=2, space="PSUM") as psA,
        tc.tile_pool(name="psB", bufs=1, space="PSUM") as psB,
    ):
        w1_t = wts.tile([48, 64], FDT, tag="w1")
        nc.sync.dma_start(out=w1_t[:], in_=w_d[0][:])
        w2_t = wts.tile([64, 64], FDT, tag="w2")
        nc.sync.dma_start(out=w2_t[:], in_=w_d[1][:])
        w3_t = wts.tile([64, 33], FDT, tag="w3")
        nc.sync.dma_start(out=w3_t[:], in_=w_d[2][:])
        b1_t = wts.tile([64, 1], FDT, tag="b1")
        nc.sync.dma_start(out=b1_t[:], in_=b_d[0][:])
        b2_t = wts.tile([64, 1], FDT, tag="b2")
        nc.sync.dma_start(out=b2_t[:], in_=b_d[1][:])
        b3_t = wts.tile([33, 1], FDT, tag="b3")
        nc.sync.dma_start(out=b3_t[:], in_=b_d[2][:])
        id_t = wts.tile([P, P], FDT, tag="id")
        nc.sync.dma_start(out=id_t[:], in_=ident_d[:])

        def body(it2):
            encc = mlp.tile([P, ch2, 48], FDT, tag="encc")
            nc.sync.dma_start(out=encc[:], in_=enc_d[:, ts(it2, ch2), :])
            outs = mlp.tile([P, ch2, 33], FDT, tag="outs")
            for nt in range(ch2 // NT):
                encT = mlp.tile([48, NT * P], FDT, tag="encT")
                for j in range(NT):
                    i = nt * NT + j
                    pt = psA.tile([48, P], FDT, tag="tp")
                    nc.tensor.transpose(out=pt[:], in_=encc[:, i, :],
                                        identity=id_t[:])
                    nc.vector.tensor_copy(out=encT[:, ts(j, P)], in_=pt[:])
                ncols = NT * P
                h1p = psB.tile([64, ncols], FDT, tag="h1p")
                nc.tensor.matmul(out=h1p[:], lhsT=w1_t[:], rhs=encT[:],
                                 start=True, stop=True)
                h1 = mlp.tile([64, ncols], FDT, tag="h1")
                nc.scalar.activation(out=h1[:], in_=h1p[:],
                                     func=mybir.ActivationFunctionType.Relu,
                                     bias=b1_t[:])
                h2p = psB.tile([64, ncols], FDT, tag="h2p")
                nc.tensor.matmul(out=h2p[:], lhsT=w2_t[:], rhs=h1[:],
                                 start=True, stop=True)
                h2 = mlp.tile([64, ncols], FDT, tag="h2")
                nc.scalar.activation(out=h2[:], in_=h2p[:],
                                     func=mybir.ActivationFunctionType.Relu,
                                     bias=b2_t[:])
                h3p = psB.tile([33, ncols], FDT, tag="h3p")
                nc.tensor.matmul(out=h3p[:], lhsT=w3_t[:], rhs=h2[:],
                                 start=True, stop=True)
                h3 = mlp.tile([33, ncols], FDT, tag="h3")
                nc.scalar.activation(out=h3[:], in_=h3p[:],
                                     func=mybir.ActivationFunctionType.Identity,
                                     bias=b3_t[:])
                # softplus(x) = ln(1 + exp(x)); logits are O(1) so exp is safe
                nc.scalar.activation(out=h3[0:1, :], in_=h3[0:1, :],
                                     func=mybir.ActivationFunctionType.Exp)
                nc.vector.tensor_scalar_add(out=h3[0:1, :], in0=h3[0:1, :], scalar1=1.0)
                nc.scalar.activation(out=h3[0:1, :], in_=h3[0:1, :],
                                     func=mybir.ActivationFunctionType.Ln)
                for j in range(NT):
                    i = nt * NT + j
                    po = psA.tile([P, 33], FDT, tag="po")
                    nc.tensor.transpose(out=po[:], in_=h3[:, ts(j, P)],
                                        identity=id_t[0:33, 0:33])
                    nc.vector.tensor_copy(out=outs[:, i, :], in_=po[:])
            nc.sync.dma_start(out=out_d[:, ts(it2, ch2), :], in_=outs[:])

        with tc.For_i(0, n2, 1) as it2:
            body(it2)


def _build_nc(cols):
    nc = bacc.Bacc("TRN2", num_devices=N_CORES)
    pts_d = nc.declare_dram_parameter("pts", [P, cols, 3], FDT, isOutput=False)
    tbl_d = nc.declare_dram_parameter("tbl", [L * T, F], FDT, isOutput=False)
    w1_d = nc.declare_dram_parameter("w1", [48, 64], FDT, isOutput=False)
    w2_d = nc.declare_dram_parameter("w2", [64, 64], FDT, isOutput=False)
    w3_d = nc.declare_dram_parameter("w3", [64, 33], FDT, isOutput=False)
    b1_d = nc.declare_dram_parameter("b1", [64, 1], FDT, isOutput=False)
    b2_d = nc.declare_dram_parameter("b2", [64, 1], FDT, isOutput=False)
    b3_d = nc.declare_dram_parameter("b3", [33, 1], FDT, isOutput=False)
    ident_d = nc.declare_dram_parameter("ident", [P, P], FDT, isOutput=False)
    res_d = nc.declare_dram_parameter("resv", [P, L], FDT, isOutput=False)
    lvb_d = nc.declare_dram_parameter("lvb", [P, L], IDT, isOutput=False)
    out_d = nc.declare_dram_parameter("out", [P, cols, 33], FDT, isOutput=True)
    enc_d = nc.dram_tensor("encbuf", [P, cols, 48], FDT)

    with tile.TileContext(nc) as tc:
        _encode_phase(nc, tc, cols, pts_d, tbl_d, res_d, lvb_d, enc_d)
    with tile.TileContext(nc) as tc:
        _mlp_phase(nc, tc, cols, enc_d, (w1_d, w2_d, w3_d), (b1_d, b2_d, b3_d),
                   ident_d, out_d)
    nc.compile()
    return nc


def _get_nc(cols):
    if cols not in _CACHE:
        _CACHE[cols] = _build_nc(cols)
    return _CACHE[cols]


def kernel(points, bb_sides, hash_table, w1, b1, w2, b2, w3, b3):
    points = np.asarray(points, np.float32)
    hash_table = np.asarray(hash_table, np.float32)
    N = points.shape[0]
    npc = N // N_CORES
    cols = npc // P
    nc = _get_nc(cols)

    common = {
        "tbl": np.ascontiguousarray(hash_table.reshape(L * T, F)),
        "w1": np.asarray(w1, np.float32),
        "w2": np.asarray(w2, np.float32),
        "w3": np.asarray(w3, np.float32),
        "b1": np.asarray(b1, np.float32).reshape(64, 1),
        "b2": np.asarray(b2, np.float32).reshape(64, 1),
        "b3": np.asarray(b3, np.float32).reshape(33, 1),
        "ident": np.eye(P, dtype=np.float32),
        "resv": np.broadcast_to(RES, (P, L)).copy(),
        "lvb": np.broadcast_to((np.arange(L, dtype=np.int64) * T).astype(np.int32),
                               (P, L)).copy(),
    }
    in_maps = []
    for c in range(N_CORES):
        shard = points[c * npc:(c + 1) * npc].reshape(P, cols, 3)
        in_maps.append({"pts": np.ascontiguousarray(shard), **common})

    res = run_bass_kernel_spmd(nc, in_maps, core_ids=list(range(N_CORES)))
    outs = [res.results[c]["out"].reshape(npc, 33) for c in range(N_CORES)]
    full = np.concatenate(outs, axis=0)
    return full[:, 0:1].copy(), full[:, 1:].copy()
